# revision 13
# baseline (speedup 1.0000x reference)
"""Trainium2 Bass kernel for nn_BioSimulatorHILO.

Strategy
--------
The reference sums per-electrode Gaussian splats over a 256x256 image:
    out[b,h,w] = clip(2 * sum_n Bv[b,n] * exp(-(dx^2+dy^2)/(2 s^2)), 0, 1)
with dx = (xs[w]-vx[n])*DEG2PIX, dy = (xs[h]-vy[n])*DEG2PIX.  The Gaussian is
separable in the pixel axes, so with
    Ex[n,w]  = exp(-((xs[w]-vx[n])*f[n])^2)           f = DEG2PIX/(sqrt2*sigma)
    EyB[n,h] = exp(-((xs[h]-vy[n])*f[n])^2 + lnBv[n])
the electrode sum becomes a matmul:  out[h,w] = sum_n EyB[n,h] * Ex[n,w].

Sharding: 8 cores = 2 batches x 4 electrode chunks (256 electrodes each; two
128-partition k-tiles).  The per-electrode scalar parameters (wedge-dipole
map -> vx,vy, sigma -> f, brightness -> lnBv) are O(N) host-side prep,
computed in numpy alongside the electrode-grid setup; the device builds the
(128,256) Ex/EyB tiles (DVE squared-distance + ACT square/exp, bf16 output)
and contracts them on the tensor engine into a (256,256) partial image per
core.  The host sums the 4 partials per batch in fp32, scales by 2, clips.

Scheduling (from NTFF profiling): the NRT preamble/postamble bracket the
body with ~14us of fixed cost, and the body is latency-bound on the pk-DMA
completion (~7.9us absolute).  Hence:
  - the Bass-init all-engine butterfly barrier is deleted (it only fences
    the framework constant-memsets, which complete ~2.5us before first use),
    so every engine flows straight from the NRT preamble into kernel work;
  - pk is DMA'd from the Scalar queue, whose stream head issues earliest,
    and the ACT table load runs right after it, all during the preamble of
    the other engines;
  - the pixel grid is generated on-device (GpSimd iota + DVE scale) instead
    of DMA'd;
  - the tensor engine runs warm-up matmuls on garbage during the DMA wait
    so the PE HAM clock-gate opens (2.4 GHz) before the real contraction;
  - PSUM->SBUF casts and output DMAs are split across ACT/DVE and
    Scalar/Sync queues to halve the tail.
A throwaway warm-up execution absorbs the one-time NRT lazy-init races
(GPSIMD library + ACT table TDRAM staging make the very first execution
produce garbage when touched this early).
"""

import sys

sys.path.insert(0, "/opt/trn_rl_repo")

import numpy as np

# ---------------------------------------------------------------- constants
GRID = 32
N = GRID * GRID
H = 256
W = 256
K_, A_, B_ = 17.3, 0.75, 120.0
SPREAD, R2S = 0.000675, 0.5
SLOPE, HALF = 19152642.5, 1.057e-07
RHEO, FREQ, PW = 2.39e-05, 300.0, 0.00017


def _compute_fov():
    xc = np.linspace(-15.0, 15.0, GRID)
    gx, gy = np.meshgrid(xc, xc, indexing="xy")
    ewk = np.exp((gx + 1j * gy) / K_)
    z = A_ * B_ * (ewk - 1.0) / (B_ - A_ * ewk)
    return float(max(np.abs(z.real).max(), np.abs(z.imag).max()) * 1.1)


FOV = _compute_fov()
DEG2PIX = H / (FOV * 2.0)
XS_STEP = 2.0 * FOV / (H - 1)

N_WARM_MM = 14  # PE warm-up matmuls (~3.2us of PE activity before the real ones)

_CACHE = {}


def _build():
    import concourse.bacc as bacc
    import concourse.mybir as mybir

    dt = mybir.dt.float32
    bf16 = mybir.dt.bfloat16
    i32 = mybir.dt.int32
    Op = mybir.AluOpType
    Act = mybir.ActivationFunctionType

    nc = bacc.Bacc(
        "TRN2",
        target_bir_lowering=False,
        debug=False,
        num_devices=8,
        # raw (non-Tile) kernel: cross-engine edges are explicitly
        # semaphored; the rust race detector has no notion of same-engine
        # program order and flags every back-to-back pair.
        detect_race_conditions=False,
    )

    # pk columns: vx0,vx1, vy0,vy1, f0,f1, nayf0,nayf1, lnBv0,lnBv1
    pk_d = nc.dram_tensor("pk", [128, 10], dt, kind="ExternalInput").ap()
    out_d = nc.dram_tensor("out", [2, 128, 256], bf16, kind="ExternalOutput").ap()

    s_pk = nc.alloc_semaphore("s_pk")
    s_g = nc.alloc_semaphore("s_g")    # iota done
    s_u = nc.alloc_semaphore("s_u")    # DVE squared-distance tiles ready
    s_a = nc.alloc_semaphore("s_a")    # ACT Ex/Ey pairs ready
    s_p = nc.alloc_semaphore("s_p")    # PE accumulation groups done
    s_x = nc.alloc_semaphore("s_x")    # xs conversion done (DVE)
    s_c0 = nc.alloc_semaphore("s_c0")  # ocp0 cast done (ACT)
    s_c1 = nc.alloc_semaphore("s_c1")  # ocp1 cast done (DVE)
    s_out = nc.alloc_semaphore("s_out")  # out-DMA completion; never waited

    def sbuf(name, cols, dtype=dt):
        return nc.alloc_sbuf_tensor(name, [128, cols], dtype).ap()

    pk = sbuf("pk_s", 10)
    xs_i = sbuf("xs_i", 256, i32)
    xs = sbuf("xs_s", 256)
    ux0 = sbuf("ux0", 256)
    ux1 = sbuf("ux1", 256)
    ux20 = sbuf("ux20", 256)
    ux21 = sbuf("ux21", 256)
    Ex0 = sbuf("Ex0", 256, bf16)
    Ex1 = sbuf("Ex1", 256, bf16)
    Ey0 = sbuf("Ey0", 256, bf16)
    Ey1 = sbuf("Ey1", 256, bf16)
    ocp0 = sbuf("ocp0", 256, bf16)
    ocp1 = sbuf("ocp1", 256, bf16)

    vx = pk[:, 0:2]
    f_t = pk[:, 4:6]
    nayf = pk[:, 6:8]
    lnBv = pk[:, 8:10]

    acc0 = nc.alloc_psum_tensor("acc0", [128, 256], dt).ap()
    acc1 = nc.alloc_psum_tensor("acc1", [128, 256], dt).ap()
    accw = nc.alloc_psum_tensor("accw", [128, 256], dt).ap()  # warm-up target
    # uy2 tiles live in PSUM: ACT is closer to PSUM, so Square-dst and
    # Exp-src both run at the (172+FD) cost instead of (224+FD).
    uy20 = nc.alloc_psum_tensor("uy20", [128, 256], dt).ap()
    uy21 = nc.alloc_psum_tensor("uy21", [128, 256], dt).ap()

    V = nc.vector
    S = nc.scalar
    G = nc.gpsimd
    SY = nc.sync
    PE = nc.tensor

    # ---------------- gpsimd: pixel-grid indices (hoisted to head) -------
    G.iota(xs_i, [[1, 256]], base=0, channel_multiplier=0).then_inc(s_g, 1)

    # ---------------- scalar: pk DMA, table load, squares + exps ---------
    # pk rides the Scalar HWDGE queue: the Scalar stream head issues ~0.6us
    # earlier than Sync's.  compile() inserts the ACT table load right
    # before the first activation, i.e. just after the DMA issue.
    # Dependent pairs are separated by at least one independent activation.
    S.dma_start(pk, pk_d).then_inc(s_pk, 16)
    S.wait_ge(s_x, 1)
    S.wait_ge(s_pk, 16)
    S.activation(uy20, xs, Act.Square, scale=f_t[:, 0:1], bias=nayf[:, 0:1])
    S.activation(uy21, xs, Act.Square, scale=f_t[:, 1:2], bias=nayf[:, 1:2])
    S.activation(Ey0, uy20, Act.Exp, scale=-1.0, bias=lnBv[:, 0:1])
    S.activation(Ey1, uy21, Act.Exp, scale=-1.0, bias=lnBv[:, 1:2])
    S.wait_ge(s_u, 1)
    S.activation(Ex0, ux20, Act.Exp, scale=-1.0).then_inc(s_a, 1)  # a=1
    S.wait_ge(s_u, 2)
    S.activation(Ex1, ux21, Act.Exp, scale=-1.0).then_inc(s_a, 1)  # a=2
    S.wait_ge(s_p, 1)
    S.activation(ocp0, acc0, Act.Copy).then_inc(s_c0, 1)  # fp32 PSUM -> bf16
    S.wait_ge(s_c1, 1)
    S.dma_start(out_d[1], ocp1).then_inc(s_out, 16)

    # ---------------- vector: xs scale, x squared-distances, cast --------
    # xs conversion waits for the iota; RAW pairs separated by one op.
    junk_v = sbuf("junk_v", 1)
    V.wait_ge(s_g, 1)
    V.tensor_scalar(xs, xs_i, XS_STEP, -FOV, Op.mult, Op.add).then_inc(s_x, 1)
    # pipeline spacer: the next op reads xs, written by the previous one
    V.tensor_scalar(junk_v, xs_i[:, 0:1], 1.0, None, Op.mult)
    V.wait_ge(s_pk, 16)
    V.tensor_scalar(ux0, xs, vx[:, 0:1], f_t[:, 0:1], Op.subtract, Op.mult)
    V.tensor_scalar(ux1, xs, vx[:, 1:2], f_t[:, 1:2], Op.subtract, Op.mult)
    V.tensor_mul(ux20, ux0, ux0).then_inc(s_u, 1)  # u=1
    V.tensor_mul(ux21, ux1, ux1).then_inc(s_u, 1)  # u=2
    V.wait_ge(s_p, 2)
    V.tensor_copy(ocp1, acc1).then_inc(s_c1, 1)  # fp32 PSUM -> bf16

    # ---------------- sync: second output DMA ----------------------------
    SY.wait_ge(s_c0, 1)
    SY.dma_start(out_d[0], ocp0).then_inc(s_out, 16)

    # ---------------- tensor: warm-up + 4 bf16 matmuls -------------------
    # The warm-up matmuls chew garbage into a scratch bank with no waits;
    # ~3.2us of continuous PE activity opens the HAM clock gate before the
    # real contraction starts.
    for _ in range(N_WARM_MM):
        PE.matmul(accw, Ey0[:, 0:128], Ex0, start=True, stop=True)
    PE.wait_ge(s_a, 1)
    PE.matmul(acc0, Ey0[:, 0:128], Ex0, start=True, stop=False)
    PE.matmul(acc1, Ey0[:, 128:256], Ex0, start=True, stop=False)
    PE.wait_ge(s_a, 2)
    PE.matmul(acc0, Ey1[:, 0:128], Ex1, start=False, stop=True).then_inc(s_p, 1)
    PE.matmul(acc1, Ey1[:, 128:256], Ex1, start=False, stop=True).then_inc(s_p, 1)

    blk = nc.main_func.blocks[0]
    insts = blk.instructions

    # ---------------- delete the Bass-init butterfly barrier -------------
    # It only fences the framework constant-memsets (GpSimd, done ~7.0us)
    # against kernel consumers (earliest ACT use ~9.5us); removing it lets
    # every engine start its kernel stream right after the NRT preamble.
    bar = set(nc.barrier_sems)

    def _touches_barrier(ins):
        si = getattr(ins, "sync_info", None)
        if si is None:
            return False
        for w in (getattr(si, "on_wait", None) or []):
            if getattr(w, "id", None) in bar:
                return True
        for u in (getattr(si, "on_update", None) or []):
            if getattr(u, "id", None) in bar:
                return True
        return False

    for ins in [i for i in insts if _touches_barrier(i)]:
        insts.remove(ins)

    # ---------------- hoist the iota to the stream head ------------------
    # (ahead of the framework memsets on GpSimd, so the DVE xs conversion
    # can run during the other engines' preamble)
    sel = [i for i in insts if type(i).__name__ == "InstIota"]
    for ins in sel:
        insts.remove(ins)
    for ins in reversed(sel):
        insts.insert(0, ins)

    nc.compile()

    # compile() put the ACT table load at the head of the Scalar stream,
    # ahead of the pk DMA issue; swap so the DMA (whose ~1.7us completion
    # latency gates everything) issues first.
    insts = nc.main_func.blocks[0].instructions
    pkdma = [
        i
        for i in insts
        if type(i).__name__ == "InstDMACopy"
        and getattr(i, "engine", None) == mybir.EngineType.Activation
        and any("pk_s" in str(o) for o in i.outs)
    ]
    for ins in pkdma:
        insts.remove(ins)
    for ins in reversed(pkdma):
        insts.insert(0, ins)
    return nc


def _get_nc():
    if "nc" not in _CACHE:
        nc = _build()
        # Throwaway execution: the very first run of a NEFF races NRT's
        # lazy staging of the GPSIMD library and ACT tables when they are
        # touched during the preamble window; one dummy execution makes
        # all subsequent runs deterministic.
        from concourse.bass_utils import run_bass_kernel_spmd

        zeros = [{"pk": np.zeros((128, 10), np.float32)} for _ in range(8)]
        run_bass_kernel_spmd(nc, zeros, list(range(8)))
        _CACHE["nc"] = nc
    return _CACHE["nc"]


def _host_params(stimulation, phi):
    """Per-electrode Gaussian parameters, mirroring the reference math."""
    f64 = np.float64
    flat = np.asarray(stimulation, f64).reshape(2, N)
    phi = np.asarray(phi, f64)

    xc = np.linspace(-15.0, 15.0, GRID)
    gx0, gy0 = np.meshgrid(xc, xc, indexing="xy")
    gxb = gx0.reshape(1, -1)
    gyb = gy0.reshape(1, -1)

    th = np.deg2rad(phi[:, 2:3])
    c, s = np.cos(th), np.sin(th)
    gx = gxb * c - gyb * s + phi[:, 0:1] * 3.5
    gy = gxb * s + gyb * c + phi[:, 1:2] * 3.5

    ewk = np.exp((gx + 1j * gy) / K_)
    z = A_ * B_ * (ewk - 1.0) / (B_ - A_ * ewk)
    vx, vy = z.real, z.imag
    r = np.abs(z)
    M = K_ * (1.0 / (r + A_) - 1.0 / (r + B_))

    sp = np.clip(phi[:, 3:4], 0.1, 10.0)
    bs = np.clip(phi[:, 4:5], 0.1, 5.0)
    zs = np.clip(phi[:, 5:6], 0.1, 5.0)
    ts = np.clip(phi[:, 6:7], 0.1, 5.0)
    cc = np.clip(phi[:, 7:8], 0.1, 5.0)

    I = flat * 8e-05
    Ieff = np.maximum(I - RHEO * ts, 0.0)
    Q = Ieff * PW * FREQ
    Bv = bs / (1.0 + np.exp(-SLOPE * (Q - HALF)))
    lnBv = np.log(Bv) / np.maximum(cc, 0.5)

    size_base = np.sqrt(I / (SPREAD * sp))
    sig = size_base * (R2S / (M + 1e-09)) * zs
    sig_px = np.maximum(sig * DEG2PIX, 1.0)
    f = DEG2PIX / (np.sqrt(2.0) * sig_px)
    return vx, vy, f, lnBv


def _make_in_maps(stimulation, phi):
    f32 = np.float32
    vx, vy, f, lnBv = _host_params(stimulation, phi)
    nayf = -vy * f

    in_maps = []
    for c in range(8):
        b, j = divmod(c, 4)
        sl = slice(j * 256, (j + 1) * 256)
        pk = np.empty((128, 10), dtype=f32)
        pk[:, 0:2] = vx[b, sl].reshape(2, 128).T
        pk[:, 2:4] = vy[b, sl].reshape(2, 128).T
        pk[:, 4:6] = f[b, sl].reshape(2, 128).T
        pk[:, 6:8] = nayf[b, sl].reshape(2, 128).T
        pk[:, 8:10] = lnBv[b, sl].reshape(2, 128).T
        in_maps.append({"pk": pk})
    return in_maps


def kernel(stimulation, phi):
    from concourse.bass_utils import run_bass_kernel_spmd

    nc = _get_nc()
    in_maps = _make_in_maps(stimulation, phi)
    res = run_bass_kernel_spmd(nc, in_maps, list(range(8))).results

    parts = np.stack(
        [np.asarray(res[c]["out"]).astype(np.float32) for c in range(8)]
    )  # (8, 2, 128, 256)
    img = parts.reshape(2, 4, 256, 256).sum(axis=1, dtype=np.float32)
    out = np.clip(img * np.float32(2.0), 0.0, 1.0).astype(np.float32)
    return out[:, None]  # (2, 1, 256, 256)


# revision 15
# speedup vs baseline: 1.1188x; 1.1188x over previous
"""Trainium2 Bass kernel for nn_BioSimulatorHILO.

Strategy
--------
The reference sums per-electrode Gaussian splats over a 256x256 image:
    out[b,h,w] = clip(2 * sum_n Bv[b,n] * exp(-(dx^2+dy^2)/(2 s^2)), 0, 1)
with dx = (xs[w]-vx[n])*DEG2PIX, dy = (xs[h]-vy[n])*DEG2PIX.  The Gaussian is
separable in the pixel axes, so with
    Ex[n,w]  = exp(-((xs[w]-vx[n])*f[n])^2)           f = DEG2PIX/(sqrt2*sigma)
    EyB[n,h] = exp(-((xs[h]-vy[n])*f[n])^2 + lnBv[n])
the electrode sum becomes a matmul:  out[h,w] = sum_n EyB[n,h] * Ex[n,w].

Sharding: 8 cores = 2 batches x 4 electrode chunks (256 electrodes each; two
128-partition k-tiles).  The per-electrode scalar parameters (wedge-dipole
map -> vx,vy, sigma -> f, brightness -> lnBv) are O(N) host-side prep,
computed in numpy alongside the electrode-grid setup; the device builds the
(128,256) Ex/EyB tiles (DVE squared-distance + ACT square/exp, bf16 output)
and contracts them on the tensor engine into a (256,256) partial image per
core.  The host sums the 4 partials per batch in fp32, scales by 2, clips.

Scheduling (from NTFF profiling): the NRT preamble/postamble bracket the
body with ~14us of fixed cost, and the body is latency-bound on the pk-DMA
completion (~7.9us absolute).  Hence:
  - the Bass-init all-engine butterfly barrier is deleted (it only fences
    the framework constant-memsets, which complete ~2.5us before first use),
    so every engine flows straight from the NRT preamble into kernel work;
  - pk is DMA'd from the Scalar queue, whose stream head issues earliest,
    and the ACT table load runs right after it, all during the preamble of
    the other engines;
  - the pixel grid is generated on-device (GpSimd iota + DVE scale) instead
    of DMA'd;
  - the tensor engine runs warm-up matmuls on garbage during the DMA wait
    so the PE HAM clock-gate opens (2.4 GHz) before the real contraction;
  - PSUM->SBUF casts and output DMAs are split across ACT/DVE and
    Scalar/Sync queues to halve the tail.
A throwaway warm-up execution absorbs the one-time NRT lazy-init races
(GPSIMD library + ACT table TDRAM staging make the very first execution
produce garbage when touched this early).
"""

import sys

sys.path.insert(0, "/opt/trn_rl_repo")

import numpy as np

# ---------------------------------------------------------------- constants
GRID = 32
N = GRID * GRID
H = 256
W = 256
K_, A_, B_ = 17.3, 0.75, 120.0
SPREAD, R2S = 0.000675, 0.5
SLOPE, HALF = 19152642.5, 1.057e-07
RHEO, FREQ, PW = 2.39e-05, 300.0, 0.00017


def _compute_fov():
    xc = np.linspace(-15.0, 15.0, GRID)
    gx, gy = np.meshgrid(xc, xc, indexing="xy")
    ewk = np.exp((gx + 1j * gy) / K_)
    z = A_ * B_ * (ewk - 1.0) / (B_ - A_ * ewk)
    return float(max(np.abs(z.real).max(), np.abs(z.imag).max()) * 1.1)


FOV = _compute_fov()
DEG2PIX = H / (FOV * 2.0)
XS_STEP = 2.0 * FOV / (H - 1)

N_WARM_MM = 38  # PE warm-up matmuls, N=128 each: fine-grained span of the DMA wait

_CACHE = {}


def _build():
    import concourse.bacc as bacc
    import concourse.mybir as mybir

    dt = mybir.dt.float32
    bf16 = mybir.dt.bfloat16
    i32 = mybir.dt.int32
    Op = mybir.AluOpType
    Act = mybir.ActivationFunctionType

    nc = bacc.Bacc(
        "TRN2",
        target_bir_lowering=False,
        debug=False,
        num_devices=8,
        # raw (non-Tile) kernel: cross-engine edges are explicitly
        # semaphored; the rust race detector has no notion of same-engine
        # program order and flags every back-to-back pair.
        detect_race_conditions=False,
    )

    # pk columns: vx0,vx1, vy0,vy1, f0,f1, nayf0,nayf1, lnBv0,lnBv1
    pk_d = nc.dram_tensor("pk", [128, 10], dt, kind="ExternalInput").ap()
    out_d = nc.dram_tensor("out", [128, 512], bf16, kind="ExternalOutput").ap()

    s_pk = nc.alloc_semaphore("s_pk")
    s_g = nc.alloc_semaphore("s_g")    # iota done
    s_u = nc.alloc_semaphore("s_u")    # DVE squared-distance tiles ready
    s_a = nc.alloc_semaphore("s_a")    # ACT Ex/Ey pairs ready
    s_p = nc.alloc_semaphore("s_p")    # PE accumulation groups done
    s_x = nc.alloc_semaphore("s_x")    # xs conversion done (DVE)
    s_c0 = nc.alloc_semaphore("s_c0")  # ocp0 cast done (ACT)
    s_c1 = nc.alloc_semaphore("s_c1")  # ocp1 cast done (DVE)
    s_out = nc.alloc_semaphore("s_out")  # out-DMA completion; never waited

    def sbuf(name, cols, dtype=dt):
        return nc.alloc_sbuf_tensor(name, [128, cols], dtype).ap()

    pk = sbuf("pk_s", 10)
    xs_i = sbuf("xs_i", 256, i32)
    xs = sbuf("xs_s", 256)
    ux0 = sbuf("ux0", 256)
    ux1 = sbuf("ux1", 256)
    ux20 = sbuf("ux20", 256)
    ux21 = sbuf("ux21", 256)
    Ex0 = sbuf("Ex0", 256, bf16)
    Ex1 = sbuf("Ex1", 256, bf16)
    Ey0 = sbuf("Ey0", 256, bf16)
    Ey1 = sbuf("Ey1", 256, bf16)
    ocp = sbuf("ocp", 512, bf16)  # [h0-half | h1-half], one contiguous out tile

    vx = pk[:, 0:2]
    f_t = pk[:, 4:6]
    nayf = pk[:, 6:8]
    lnBv = pk[:, 8:10]

    acc0 = nc.alloc_psum_tensor("acc0", [128, 256], dt).ap()
    acc1 = nc.alloc_psum_tensor("acc1", [128, 256], dt).ap()
    accw = nc.alloc_psum_tensor("accw", [128, 256], dt).ap()  # warm-up target
    # uy2 tiles live in PSUM: ACT is closer to PSUM, so Square-dst and
    # Exp-src both run at the (172+FD) cost instead of (224+FD).
    uy20 = nc.alloc_psum_tensor("uy20", [128, 256], dt).ap()
    uy21 = nc.alloc_psum_tensor("uy21", [128, 256], dt).ap()

    V = nc.vector
    S = nc.scalar
    G = nc.gpsimd
    SY = nc.sync
    PE = nc.tensor

    # ---------------- gpsimd: pixel-grid indices (hoisted to head) -------
    G.iota(xs_i, [[1, 256]], base=0, channel_multiplier=0).then_inc(s_g, 1)

    # ---------------- scalar: pk DMA, table load, squares + exps ---------
    # pk rides the Scalar HWDGE queue: the Scalar stream head issues ~0.6us
    # earlier than Sync's.  compile() inserts the ACT table load right
    # before the first activation, i.e. just after the DMA issue.
    # Dependent pairs are separated by at least one independent activation.
    S.dma_start(pk, pk_d).then_inc(s_pk, 16)
    S.wait_ge(s_x, 1)
    S.wait_ge(s_pk, 16)
    S.activation(uy20, xs, Act.Square, scale=f_t[:, 0:1], bias=nayf[:, 0:1])
    S.activation(uy21, xs, Act.Square, scale=f_t[:, 1:2], bias=nayf[:, 1:2])
    S.activation(Ey0, uy20, Act.Exp, scale=-1.0, bias=lnBv[:, 0:1])
    S.activation(Ey1, uy21, Act.Exp, scale=-1.0, bias=lnBv[:, 1:2])
    S.wait_ge(s_u, 1)
    S.activation(Ex0, ux20, Act.Exp, scale=-1.0).then_inc(s_a, 1)  # a=1
    S.wait_ge(s_u, 2)
    S.activation(Ex1, ux21, Act.Exp, scale=-1.0).then_inc(s_a, 1)  # a=2
    S.wait_ge(s_p, 1)
    S.activation(ocp[:, 0:256], acc0, Act.Copy).then_inc(s_c0, 1)  # PSUM -> bf16

    # ---------------- vector: xs scale, x squared-distances, cast --------
    # xs conversion waits for the iota; RAW pairs separated by one op.
    junk_v = sbuf("junk_v", 1)
    V.wait_ge(s_g, 1)
    V.tensor_scalar(xs, xs_i, XS_STEP, -FOV, Op.mult, Op.add).then_inc(s_x, 1)
    # pipeline spacer: the next op reads xs, written by the previous one
    V.tensor_scalar(junk_v, xs_i[:, 0:1], 1.0, None, Op.mult)
    V.wait_ge(s_pk, 16)
    V.tensor_scalar(ux0, xs, vx[:, 0:1], f_t[:, 0:1], Op.subtract, Op.mult)
    V.tensor_scalar(ux1, xs, vx[:, 1:2], f_t[:, 1:2], Op.subtract, Op.mult)
    V.tensor_mul(ux20, ux0, ux0).then_inc(s_u, 1)  # u=1
    V.tensor_mul(ux21, ux1, ux1).then_inc(s_u, 1)  # u=2
    V.wait_ge(s_p, 2)
    V.tensor_copy(ocp[:, 256:512], acc1).then_inc(s_c1, 1)  # fp32 PSUM -> bf16

    # ---------------- sync: single merged output DMA ----------------------
    SY.wait_ge(s_c0, 1)
    SY.wait_ge(s_c1, 1)
    SY.dma_start(out_d, ocp).then_inc(s_out, 16)

    # ---------------- tensor: warm-up + 4 bf16 matmuls -------------------
    # The warm-up matmuls chew garbage into a scratch bank with no waits;
    # ~3.2us of continuous PE activity opens the HAM clock gate before the
    # real contraction starts.
    for _ in range(N_WARM_MM):
        PE.matmul(accw[:, 0:128], Ey0[:, 0:128], Ex0[:, 0:128], start=True, stop=True)
    PE.wait_ge(s_a, 1)
    PE.matmul(acc0, Ey0[:, 0:128], Ex0, start=True, stop=False)
    PE.matmul(acc1, Ey0[:, 128:256], Ex0, start=True, stop=False)
    PE.wait_ge(s_a, 2)
    PE.matmul(acc0, Ey1[:, 0:128], Ex1, start=False, stop=True).then_inc(s_p, 1)
    PE.matmul(acc1, Ey1[:, 128:256], Ex1, start=False, stop=True).then_inc(s_p, 1)

    blk = nc.main_func.blocks[0]
    insts = blk.instructions

    # ---------------- delete the Bass-init butterfly barrier -------------
    # It only fences the framework constant-memsets (GpSimd, done ~7.0us)
    # against kernel consumers (earliest ACT use ~9.5us); removing it lets
    # every engine start its kernel stream right after the NRT preamble.
    bar = set(nc.barrier_sems)

    def _touches_barrier(ins):
        si = getattr(ins, "sync_info", None)
        if si is None:
            return False
        for w in (getattr(si, "on_wait", None) or []):
            if getattr(w, "id", None) in bar:
                return True
        for u in (getattr(si, "on_update", None) or []):
            if getattr(u, "id", None) in bar:
                return True
        return False

    for ins in [i for i in insts if _touches_barrier(i)]:
        insts.remove(ins)

    # ---------------- hoist the iota to the stream head ------------------
    # (ahead of the framework memsets on GpSimd, so the DVE xs conversion
    # can run during the other engines' preamble)
    sel = [i for i in insts if type(i).__name__ == "InstIota"]
    for ins in sel:
        insts.remove(ins)
    for ins in reversed(sel):
        insts.insert(0, ins)

    nc.compile()

    # compile() put the ACT table load at the head of the Scalar stream,
    # ahead of the pk DMA issue; swap so the DMA (whose ~1.7us completion
    # latency gates everything) issues first.
    insts = nc.main_func.blocks[0].instructions
    pkdma = [
        i
        for i in insts
        if type(i).__name__ == "InstDMACopy"
        and getattr(i, "engine", None) == mybir.EngineType.Activation
        and any("pk_s" in str(o) for o in i.outs)
    ]
    for ins in pkdma:
        insts.remove(ins)
    for ins in reversed(pkdma):
        insts.insert(0, ins)
    return nc


def _get_nc():
    if "nc" not in _CACHE:
        nc = _build()
        # Throwaway execution: the very first run of a NEFF races NRT's
        # lazy staging of the GPSIMD library and ACT tables when they are
        # touched during the preamble window; one dummy execution makes
        # all subsequent runs deterministic.
        from concourse.bass_utils import run_bass_kernel_spmd

        zeros = [{"pk": np.zeros((128, 10), np.float32)} for _ in range(8)]
        run_bass_kernel_spmd(nc, zeros, list(range(8)))
        _CACHE["nc"] = nc
    return _CACHE["nc"]


def _host_params(stimulation, phi):
    """Per-electrode Gaussian parameters, mirroring the reference math."""
    f64 = np.float64
    flat = np.asarray(stimulation, f64).reshape(2, N)
    phi = np.asarray(phi, f64)

    xc = np.linspace(-15.0, 15.0, GRID)
    gx0, gy0 = np.meshgrid(xc, xc, indexing="xy")
    gxb = gx0.reshape(1, -1)
    gyb = gy0.reshape(1, -1)

    th = np.deg2rad(phi[:, 2:3])
    c, s = np.cos(th), np.sin(th)
    gx = gxb * c - gyb * s + phi[:, 0:1] * 3.5
    gy = gxb * s + gyb * c + phi[:, 1:2] * 3.5

    ewk = np.exp((gx + 1j * gy) / K_)
    z = A_ * B_ * (ewk - 1.0) / (B_ - A_ * ewk)
    vx, vy = z.real, z.imag
    r = np.abs(z)
    M = K_ * (1.0 / (r + A_) - 1.0 / (r + B_))

    sp = np.clip(phi[:, 3:4], 0.1, 10.0)
    bs = np.clip(phi[:, 4:5], 0.1, 5.0)
    zs = np.clip(phi[:, 5:6], 0.1, 5.0)
    ts = np.clip(phi[:, 6:7], 0.1, 5.0)
    cc = np.clip(phi[:, 7:8], 0.1, 5.0)

    I = flat * 8e-05
    Ieff = np.maximum(I - RHEO * ts, 0.0)
    Q = Ieff * PW * FREQ
    Bv = bs / (1.0 + np.exp(-SLOPE * (Q - HALF)))
    lnBv = np.log(Bv) / np.maximum(cc, 0.5)

    size_base = np.sqrt(I / (SPREAD * sp))
    sig = size_base * (R2S / (M + 1e-09)) * zs
    sig_px = np.maximum(sig * DEG2PIX, 1.0)
    f = DEG2PIX / (np.sqrt(2.0) * sig_px)
    return vx, vy, f, lnBv


def _make_in_maps(stimulation, phi):
    f32 = np.float32
    vx, vy, f, lnBv = _host_params(stimulation, phi)
    nayf = -vy * f

    in_maps = []
    for c in range(8):
        b, j = divmod(c, 4)
        sl = slice(j * 256, (j + 1) * 256)
        pk = np.empty((128, 10), dtype=f32)
        pk[:, 0:2] = vx[b, sl].reshape(2, 128).T
        pk[:, 2:4] = vy[b, sl].reshape(2, 128).T
        pk[:, 4:6] = f[b, sl].reshape(2, 128).T
        pk[:, 6:8] = nayf[b, sl].reshape(2, 128).T
        pk[:, 8:10] = lnBv[b, sl].reshape(2, 128).T
        in_maps.append({"pk": pk})
    return in_maps


def kernel(stimulation, phi):
    from concourse.bass_utils import run_bass_kernel_spmd

    nc = _get_nc()
    in_maps = _make_in_maps(stimulation, phi)
    res = run_bass_kernel_spmd(nc, in_maps, list(range(8))).results

    parts = np.stack(
        [np.asarray(res[c]["out"]).astype(np.float32) for c in range(8)]
    )  # (8, 128, 512): [h0-half | h1-half] per core
    imgs = np.concatenate([parts[:, :, 0:256], parts[:, :, 256:512]], axis=1)
    img = imgs.reshape(2, 4, 256, 256).sum(axis=1, dtype=np.float32)
    out = np.clip(img * np.float32(2.0), 0.0, 1.0).astype(np.float32)
    return out[:, None]  # (2, 1, 256, 256)
